# revision 4
# baseline (speedup 1.0000x reference)
"""TRN2 Bass kernel for nn_BasicEuclideanDistModel (temporal point-process loss).

Strategy (data-parallel over 8 NeuronCores, per sharding hint):
  - Host: shard the 8M events / 500K sampled pairs across 8 cores, and for
    each event/pair slice out the per-node embedding rows (z0/v0 indexed by
    u, v) as bf16 streams, laid out [tiles, 128, n] for direct DMA.
    The host does no model arithmetic - only index-gather, dtype cast,
    reshape, and replication of the tiny scalar params.
  - Device (per core): stream the event arrays tile-by-tile and compute
      d_e   = sqrt(((zu-zv+eps) + (vu-vv)*t)^2 summed over x,y)  -> sum d_e
      per pair: A=|dz+eps|^2, B=2(dz+eps).dv, C=|dv|^2, and for each
      Riemann midpoint tau: q = A + B*tau + C*tau^2,
      accumulate exp(beta - sqrt(q)).
    VectorE does the elementwise math in bf16 (2x mode), ScalarE does
    sqrt/exp with fused per-partition accumulation, DMA streams at
    HBM rate. Partial sums [128,16] f32 per core are combined on host.
"""
import sys
import os
import numpy as np

sys.path.insert(0, "/opt/trn_rl_repo")

import ml_dtypes  # noqa: E402

BF16 = ml_dtypes.bfloat16

N_POINTS = 100000
N_EVENTS = 8000000
N_PAIRS = 500000
R = 10
EPS = 1e-6
N_CORES = 8

E_CORE = N_EVENTS // N_CORES          # 1,000,000
EV_TILES = 4
EV_N = 1954                           # free elems per tile
E_PAD = EV_TILES * 128 * EV_N         # 1,000,448 (448 zero-pad events/core)
P_CORE = N_PAIRS // N_CORES           # 62,500
PR_N = (P_CORE + 127) // 128          # 489 (padded)
PR_PAD = PR_N * 128 - P_CORE          # 92 pad pairs
PAD_SENTINEL = -200.0                 # pad zv_x -> d=200 -> exp(beta-200)=0

_NC_CACHE = {}


def build_nc(passes=1):
    """Build the per-core Bass program. passes>1 wraps the whole body in a
    hardware loop (same work repeated) - used only for wall-clock timing."""
    if passes in _NC_CACHE:
        return _NC_CACHE[passes]
    import concourse.bacc as bacc
    import concourse.mybir as mybir
    import concourse.tile as tile

    f32 = mybir.dt.float32
    bf16 = mybir.dt.bfloat16
    Alu = mybir.AluOpType
    Act = mybir.ActivationFunctionType

    nc = bacc.Bacc(trn_type="TRN2")

    ev_names = ["ev_zux", "ev_zuy", "ev_zvx", "ev_zvy",
                "ev_vux", "ev_vuy", "ev_vvx", "ev_vvy", "ev_t"]
    ev_dram = {n: nc.dram_tensor(n, [EV_TILES, 128, EV_N], bf16,
                                 kind="ExternalInput") for n in ev_names}
    pr_names = ["pr_zux", "pr_zuy", "pr_zvx", "pr_zvy",
                "pr_vux", "pr_vuy", "pr_vvx", "pr_vvy"]
    pr_dram = {n: nc.dram_tensor(n, [128, PR_N], bf16, kind="ExternalInput")
               for n in pr_names}
    taus_dram = nc.dram_tensor("taus", [128, R], f32, kind="ExternalInput")
    beta_dram = nc.dram_tensor("betab", [128, 1], f32, kind="ExternalInput")
    out_dram = nc.dram_tensor("partials", [128, 16], f32, kind="ExternalOutput")

    with tile.TileContext(nc) as tc:
        with (
            tc.tile_pool(name="evin", bufs=2) as evin,
            tc.tile_pool(name="work", bufs=2) as work,
            tc.tile_pool(name="prp", bufs=1) as prp,
            tc.tile_pool(name="accp", bufs=1) as accp,
        ):
            acc = accp.tile([128, 16], f32)
            taus = accp.tile([128, R], f32)
            betab = accp.tile([128, 1], f32)

            def body():
                nc.vector.memset(acc[:], 0.0)
                nc.sync.dma_start(taus[:], taus_dram.ap()[:])
                nc.sync.dma_start(betab[:], beta_dram.ap()[:])

                # ---------------- event term ----------------
                for t in range(EV_TILES):
                    tl = {n: evin.tile([128, EV_N], bf16, tag=n, name=n) for n in ev_names}
                    for n in ev_names:
                        nc.sync.dma_start(tl[n][:], ev_dram[n].ap()[t])
                    dvx = work.tile([128, EV_N], bf16, tag="dvx")
                    dvy = work.tile([128, EV_N], bf16, tag="dvy")
                    nc.vector.tensor_tensor(dvx[:], tl["ev_vux"][:], tl["ev_vvx"][:], Alu.subtract)
                    nc.vector.tensor_tensor(dvy[:], tl["ev_vuy"][:], tl["ev_vvy"][:], Alu.subtract)
                    tvx = work.tile([128, EV_N], bf16, tag="tvx")
                    tvy = work.tile([128, EV_N], bf16, tag="tvy")
                    nc.vector.tensor_tensor(tvx[:], dvx[:], tl["ev_t"][:], Alu.mult)
                    nc.vector.tensor_tensor(tvy[:], dvy[:], tl["ev_t"][:], Alu.mult)
                    dzx = work.tile([128, EV_N], bf16, tag="dzx")
                    dzy = work.tile([128, EV_N], bf16, tag="dzy")
                    nc.vector.scalar_tensor_tensor(dzx[:], tl["ev_zux"][:], EPS, tl["ev_zvx"][:], Alu.add, Alu.subtract)
                    nc.vector.scalar_tensor_tensor(dzy[:], tl["ev_zuy"][:], EPS, tl["ev_zvy"][:], Alu.add, Alu.subtract)
                    ax = work.tile([128, EV_N], bf16, tag="ax")
                    ay = work.tile([128, EV_N], bf16, tag="ay")
                    nc.vector.tensor_tensor(ax[:], dzx[:], tvx[:], Alu.add)
                    nc.vector.tensor_tensor(ay[:], dzy[:], tvy[:], Alu.add)
                    axs = work.tile([128, EV_N], bf16, tag="axs")
                    ays = work.tile([128, EV_N], bf16, tag="ays")
                    nc.vector.tensor_tensor(axs[:], ax[:], ax[:], Alu.mult)
                    nc.vector.tensor_tensor(ays[:], ay[:], ay[:], Alu.mult)
                    q = work.tile([128, EV_N], bf16, tag="q")
                    nc.vector.tensor_tensor(q[:], axs[:], ays[:], Alu.add)
                    d = work.tile([128, EV_N], bf16, tag="d")
                    nc.scalar.activation(d[:], q[:], Act.Sqrt,
                                         accum_out=acc[:, t:t + 1])

                # ---------------- non-event (pair) term ----------------
                pl = {n: prp.tile([128, PR_N], bf16, tag=n, name=n) for n in pr_names}
                for n in pr_names:
                    nc.sync.dma_start(pl[n][:], pr_dram[n].ap()[:])
                pdzx = prp.tile([128, PR_N], bf16)
                pdzy = prp.tile([128, PR_N], bf16)
                nc.vector.scalar_tensor_tensor(pdzx[:], pl["pr_zux"][:], EPS, pl["pr_zvx"][:], Alu.add, Alu.subtract)
                nc.vector.scalar_tensor_tensor(pdzy[:], pl["pr_zuy"][:], EPS, pl["pr_zvy"][:], Alu.add, Alu.subtract)
                pdvx = prp.tile([128, PR_N], bf16)
                pdvy = prp.tile([128, PR_N], bf16)
                nc.vector.tensor_tensor(pdvx[:], pl["pr_vux"][:], pl["pr_vvx"][:], Alu.subtract)
                nc.vector.tensor_tensor(pdvy[:], pl["pr_vuy"][:], pl["pr_vvy"][:], Alu.subtract)
                t1 = prp.tile([128, PR_N], bf16)
                t2 = prp.tile([128, PR_N], bf16)
                A = prp.tile([128, PR_N], bf16)
                B = prp.tile([128, PR_N], bf16)
                C = prp.tile([128, PR_N], bf16)
                nc.vector.tensor_tensor(t1[:], pdzx[:], pdzx[:], Alu.mult)
                nc.vector.tensor_tensor(t2[:], pdzy[:], pdzy[:], Alu.mult)
                nc.vector.tensor_tensor(A[:], t1[:], t2[:], Alu.add)
                nc.vector.tensor_tensor(t1[:], pdvx[:], pdvx[:], Alu.mult)
                nc.vector.tensor_tensor(t2[:], pdvy[:], pdvy[:], Alu.mult)
                nc.vector.tensor_tensor(C[:], t1[:], t2[:], Alu.add)
                nc.vector.tensor_tensor(t1[:], pdzx[:], pdvx[:], Alu.mult)
                nc.vector.tensor_tensor(t2[:], pdzy[:], pdvy[:], Alu.mult)
                nc.vector.tensor_tensor(B[:], t1[:], t2[:], Alu.add)
                nc.vector.tensor_scalar(B[:], B[:], 2.0, None, Alu.mult)
                for r in range(R):
                    s1 = work.tile([128, PR_N], bf16, tag="s1")
                    qr = work.tile([128, PR_N], bf16, tag="qr")
                    dr = work.tile([128, PR_N], bf16, tag="dr")
                    er = work.tile([128, PR_N], bf16, tag="er")
                    nc.vector.scalar_tensor_tensor(s1[:], C[:], taus[:, r:r + 1], B[:], Alu.mult, Alu.add)
                    nc.vector.scalar_tensor_tensor(qr[:], s1[:], taus[:, r:r + 1], A[:], Alu.mult, Alu.add)
                    nc.vector.tensor_scalar_max(qr[:], qr[:], 0.0)
                    nc.scalar.activation(dr[:], qr[:], Act.Sqrt)
                    nc.scalar.activation(er[:], dr[:], Act.Exp,
                                         bias=betab[:, 0:1], scale=-1.0,
                                         accum_out=acc[:, 4 + r:5 + r])
                nc.sync.dma_start(out_dram.ap()[:], acc[:])

            if passes == 1:
                body()
            else:
                with tc.For_i(0, passes):
                    body()
    nc.finalize()
    _NC_CACHE[passes] = nc
    return nc


def _host_prepare(beta, z0, v0, u, v, event_times, nu, nv, t0, tn):
    """Shard + gather inputs into per-core DMA-ready arrays (no model math)."""
    zb = np.asarray(z0, dtype=np.float32).astype(BF16)
    vb = np.asarray(v0, dtype=np.float32).astype(BF16)
    # gather via uint16 views (fast paths in numpy)
    zbu = zb.view(np.uint16)
    vbu = vb.view(np.uint16)
    u = np.asarray(u).astype(np.int64, copy=False)
    v = np.asarray(v).astype(np.int64, copy=False)
    nu = np.asarray(nu).astype(np.int64, copy=False)
    nv = np.asarray(nv).astype(np.int64, copy=False)
    tarr = np.asarray(event_times, dtype=np.float32).astype(BF16).view(np.uint16)

    t0f = float(np.asarray(t0)); tnf = float(np.asarray(tn))
    dt = (tnf - t0f) / R
    taus = (t0f + (np.arange(R, dtype=np.float64) + 0.5) * dt).astype(np.float32)
    taus_arr = np.broadcast_to(taus[None, :], (128, R)).copy()
    betaf = float(np.asarray(beta).reshape(-1)[0])
    beta_arr = np.full((128, 1), betaf, dtype=np.float32)

    in_maps = []
    for c in range(N_CORES):
        es = slice(c * E_CORE, (c + 1) * E_CORE)
        ps = slice(c * P_CORE, (c + 1) * P_CORE)
        uc, vc = u[es], v[es]
        nuc, nvc = nu[ps], nv[ps]
        m = {}
        ev_shape = (EV_TILES, 128, EV_N)

        def ev_pad(arr):
            out = np.zeros(E_PAD, dtype=np.uint16)
            out[:E_CORE] = arr
            return out.reshape(ev_shape)

        m["ev_zux"] = ev_pad(zbu[uc, 0])
        m["ev_zuy"] = ev_pad(zbu[uc, 1])
        m["ev_zvx"] = ev_pad(zbu[vc, 0])
        m["ev_zvy"] = ev_pad(zbu[vc, 1])
        m["ev_vux"] = ev_pad(vbu[uc, 0])
        m["ev_vuy"] = ev_pad(vbu[uc, 1])
        m["ev_vvx"] = ev_pad(vbu[vc, 0])
        m["ev_vvy"] = ev_pad(vbu[vc, 1])
        m["ev_t"] = ev_pad(tarr[es])

        def pr_pad(arr, fill=0):
            out = np.full(PR_N * 128, fill, dtype=np.uint16)
            out[:P_CORE] = arr
            return out.reshape(128, PR_N)

        sent = np.float32(PAD_SENTINEL).astype(BF16).view(np.uint16)[()] \
            if False else np.array(PAD_SENTINEL, dtype=np.float32).astype(BF16).view(np.uint16).item()
        m["pr_zux"] = pr_pad(zbu[nuc, 0])
        m["pr_zuy"] = pr_pad(zbu[nuc, 1])
        m["pr_zvx"] = pr_pad(zbu[nvc, 0], fill=sent)
        m["pr_zvy"] = pr_pad(zbu[nvc, 1])
        m["pr_vux"] = pr_pad(vbu[nuc, 0])
        m["pr_vuy"] = pr_pad(vbu[nuc, 1])
        m["pr_vvx"] = pr_pad(vbu[nvc, 0])
        m["pr_vvy"] = pr_pad(vbu[nvc, 1])
        # view uint16 arrays back as bf16 for the runner's dtype check
        for k in m:
            m[k] = m[k].view(BF16)
        m["taus"] = taus_arr
        m["betab"] = beta_arr
        in_maps.append(m)
    return in_maps, betaf, dt


def _combine(results, betaf, dt):
    d_sum = 0.0
    e_sum = 0.0
    for res in results:
        p = res["partials"].astype(np.float64)
        d_sum += p[:, 0:4].sum()
        e_sum += p[:, 4:4 + R].sum()
    val = N_EVENTS * float(betaf) - d_sum - e_sum * dt
    return np.array([[val]], dtype=np.float32)


def kernel(beta, z0, v0, u, v, event_times, nu, nv, t0, tn):
    from concourse import bass_utils
    in_maps, betaf, dt = _host_prepare(beta, z0, v0, u, v, event_times,
                                       nu, nv, t0, tn)
    nc = build_nc(passes=1)
    res = bass_utils.run_bass_kernel_spmd(nc, in_maps,
                                          core_ids=list(range(N_CORES)))
    return _combine(res.results, betaf, dt)


# revision 5
# speedup vs baseline: 1.7779x; 1.7779x over previous
"""TRN2 Bass kernel for nn_BasicEuclideanDistModel (temporal point-process loss).

Strategy (data-parallel over 8 NeuronCores, per sharding hint):
  - Host: shard the 8M events / 500K sampled pairs across 8 cores, and for
    each event/pair slice out the per-node embedding rows (z0/v0 indexed by
    u, v) as bf16 streams laid out [tiles, 128, 9, n] for direct DMA.
    The host does no model arithmetic - only index-gather, dtype cast,
    reshape, and replication of the tiny scalar params. (eps is folded into
    the u-side z table: one 100K-row constant shift, part of table prep.)
  - Device (per core): stream the event arrays tile-by-tile:
      a_c = (zu_c + eps - zv_c) + (vu_c - vv_c)*t   (c = x,y)  [VectorE bf16]
      sq_c = a_c^2                                   [ScalarE Square]
      d = sqrt(sq_x + sq_y), accumulated per partition [ScalarE Sqrt+accum]
    Pairs: A=|dz+eps|^2, B=2(dz+eps).dv, C=|dv|^2; per Riemann midpoint tau
      q = (C*tau + B)*tau + A (clamped >= 0), then after ALL sqrts are done,
      10x exp(beta - d) with accumulation -- sqrt/exp grouped to avoid ACT
      table-set switches (~2.7us each).
    Partial sums land in [128,16] f32 per core; host combines in f64.
"""
import sys
import os
import numpy as np

sys.path.insert(0, "/opt/trn_rl_repo")

import ml_dtypes  # noqa: E402

BF16 = ml_dtypes.bfloat16

N_POINTS = 100000
N_EVENTS = 8000000
N_PAIRS = 500000
R = 10
EPS = 1e-6
N_CORES = 8

E_CORE = N_EVENTS // N_CORES          # 1,000,000
EV_TILES = 4
EV_N = 1954                           # free elems per tile
E_PAD = EV_TILES * 128 * EV_N         # 1,000,448 (448 zero-pad events/core)
NEV = 9                               # event streams
P_CORE = N_PAIRS // N_CORES           # 62,500
PR_N = (P_CORE + 127) // 128          # 489 (padded)
NPR = 8                               # pair streams
PAD_SENTINEL = -200.0                 # pad zv_x -> d=200 -> exp(beta-200)=0

# stream order inside the packed arrays
EV_S = {n: i for i, n in enumerate(
    ["zux", "zuy", "zvx", "zvy", "vux", "vuy", "vvx", "vvy", "t"])}
PR_S = {n: i for i, n in enumerate(
    ["zux", "zuy", "zvx", "zvy", "vux", "vuy", "vvx", "vvy"])}

_NC_CACHE = {}


def build_nc(passes=1):
    """Build the per-core Bass program. passes>1 wraps the body in a
    hardware loop (same work repeated) - used only for wall-clock timing."""
    if passes in _NC_CACHE:
        return _NC_CACHE[passes]
    import concourse.bacc as bacc
    import concourse.mybir as mybir
    import concourse.tile as tile

    f32 = mybir.dt.float32
    bf16 = mybir.dt.bfloat16
    Alu = mybir.AluOpType
    Act = mybir.ActivationFunctionType

    nc = bacc.Bacc(trn_type="TRN2")

    ev_dram = nc.dram_tensor("ev_all", [EV_TILES, 128, NEV, EV_N], bf16,
                             kind="ExternalInput")
    pr_dram = nc.dram_tensor("pr_all", [128, NPR, PR_N], bf16,
                             kind="ExternalInput")
    taus_dram = nc.dram_tensor("taus", [128, R], f32, kind="ExternalInput")
    beta_dram = nc.dram_tensor("betab", [128, 1], f32, kind="ExternalInput")
    out_dram = nc.dram_tensor("partials", [128, 16], f32, kind="ExternalOutput")

    with tile.TileContext(nc) as tc:
        with (
            tc.tile_pool(name="evin", bufs=2) as evin,
            tc.tile_pool(name="work", bufs=2) as work,
            tc.tile_pool(name="prp", bufs=1) as prp,
            tc.tile_pool(name="accp", bufs=1) as accp,
        ):
            acc = accp.tile([128, 16], f32)
            taus = accp.tile([128, R], f32)
            betab = accp.tile([128, 1], f32)

            def body():
                nc.vector.memset(acc[:], 0.0)
                nc.sync.dma_start(taus[:], taus_dram.ap()[:])
                nc.sync.dma_start(betab[:], beta_dram.ap()[:])

                # -------- pair inputs + DVE prep (issued early) --------
                prt = prp.tile([128, NPR, PR_N], bf16, name="prt")
                nc.sync.dma_start(prt[:], pr_dram.ap()[:])

                def ps(n):
                    return prt[:, PR_S[n], :]

                pdzx = prp.tile([128, PR_N], bf16, name="pdzx")
                pdzy = prp.tile([128, PR_N], bf16, name="pdzy")
                pdvx = prp.tile([128, PR_N], bf16, name="pdvx")
                pdvy = prp.tile([128, PR_N], bf16, name="pdvy")
                nc.vector.tensor_tensor(pdzx[:], ps("zux"), ps("zvx"), Alu.subtract)
                nc.vector.tensor_tensor(pdzy[:], ps("zuy"), ps("zvy"), Alu.subtract)
                nc.vector.tensor_tensor(pdvx[:], ps("vux"), ps("vvx"), Alu.subtract)
                nc.vector.tensor_tensor(pdvy[:], ps("vuy"), ps("vvy"), Alu.subtract)
                t1 = prp.tile([128, PR_N], bf16, name="t1")
                t2 = prp.tile([128, PR_N], bf16, name="t2")
                A = prp.tile([128, PR_N], bf16, name="A")
                B = prp.tile([128, PR_N], bf16, name="B")
                C = prp.tile([128, PR_N], bf16, name="C")
                nc.vector.tensor_tensor(t1[:], pdzx[:], pdzx[:], Alu.mult)
                nc.vector.tensor_tensor(t2[:], pdzy[:], pdzy[:], Alu.mult)
                nc.vector.tensor_tensor(A[:], t1[:], t2[:], Alu.add)
                nc.vector.tensor_tensor(t1[:], pdvx[:], pdvx[:], Alu.mult)
                nc.vector.tensor_tensor(t2[:], pdvy[:], pdvy[:], Alu.mult)
                nc.vector.tensor_tensor(C[:], t1[:], t2[:], Alu.add)
                nc.vector.tensor_tensor(t1[:], pdzx[:], pdvx[:], Alu.mult)
                nc.vector.tensor_tensor(t2[:], pdzy[:], pdvy[:], Alu.mult)
                nc.vector.tensor_tensor(B[:], t1[:], t2[:], Alu.add)
                nc.vector.tensor_scalar(B[:], B[:], 2.0, None, Alu.mult)
                qrs = []
                for r in range(R):
                    s1 = prp.tile([128, PR_N], bf16, name=f"s1_{r}", tag="s1")
                    qr = prp.tile([128, PR_N], bf16, name=f"qr_{r}")
                    nc.vector.scalar_tensor_tensor(s1[:], C[:], taus[:, r:r + 1], B[:], Alu.mult, Alu.add)
                    nc.vector.scalar_tensor_tensor(qr[:], s1[:], taus[:, r:r + 1], A[:], Alu.mult, Alu.add)
                    nc.vector.tensor_scalar_max(qr[:], qr[:], 0.0)
                    qrs.append(qr)

                # -------- event tiles (Sqrt table set stays loaded) --------
                for t in range(EV_TILES):
                    evt = evin.tile([128, NEV, EV_N], bf16, tag="evt", name="evt")
                    nc.sync.dma_start(evt[:], ev_dram.ap()[t])

                    def es(n):
                        return evt[:, EV_S[n], :]

                    dvx = work.tile([128, EV_N], bf16, tag="dvx", name="dvx")
                    dvy = work.tile([128, EV_N], bf16, tag="dvy", name="dvy")
                    nc.vector.tensor_tensor(dvx[:], es("vux"), es("vvx"), Alu.subtract)
                    nc.vector.tensor_tensor(dvy[:], es("vuy"), es("vvy"), Alu.subtract)
                    tvx = work.tile([128, EV_N], bf16, tag="tvx", name="tvx")
                    tvy = work.tile([128, EV_N], bf16, tag="tvy", name="tvy")
                    nc.vector.tensor_tensor(tvx[:], dvx[:], es("t"), Alu.mult)
                    nc.vector.tensor_tensor(tvy[:], dvy[:], es("t"), Alu.mult)
                    dzx = work.tile([128, EV_N], bf16, tag="dzx", name="dzx")
                    dzy = work.tile([128, EV_N], bf16, tag="dzy", name="dzy")
                    # eps already folded into zu on host
                    nc.vector.tensor_tensor(dzx[:], es("zux"), es("zvx"), Alu.subtract)
                    nc.vector.tensor_tensor(dzy[:], es("zuy"), es("zvy"), Alu.subtract)
                    ax = work.tile([128, EV_N], bf16, tag="ax", name="ax")
                    ay = work.tile([128, EV_N], bf16, tag="ay", name="ay")
                    nc.vector.tensor_tensor(ax[:], dzx[:], tvx[:], Alu.add)
                    nc.vector.tensor_tensor(ay[:], dzy[:], tvy[:], Alu.add)
                    # squares on ScalarE (Square is in every ACT table set)
                    axs = work.tile([128, EV_N], bf16, tag="axs", name="axs")
                    ays = work.tile([128, EV_N], bf16, tag="ays", name="ays")
                    nc.scalar.activation(axs[:], ax[:], Act.Square)
                    nc.scalar.activation(ays[:], ay[:], Act.Square)
                    q = work.tile([128, EV_N], bf16, tag="q", name="q")
                    nc.vector.tensor_tensor(q[:], axs[:], ays[:], Alu.add)
                    d = work.tile([128, EV_N], bf16, tag="d", name="d")
                    nc.scalar.activation(d[:], q[:], Act.Sqrt,
                                         accum_out=acc[:, t:t + 1])

                # -------- pair sqrts (same table set), then all exps --------
                drs = []
                for r in range(R):
                    dr = prp.tile([128, PR_N], bf16, name=f"dr_{r}")
                    nc.scalar.activation(dr[:], qrs[r][:], Act.Sqrt)
                    drs.append(dr)
                for r in range(R):
                    er = work.tile([128, PR_N], bf16, tag="er", name="er")
                    nc.scalar.activation(er[:], drs[r][:], Act.Exp,
                                         bias=betab[:, 0:1], scale=-1.0,
                                         accum_out=acc[:, 4 + r:5 + r])
                nc.sync.dma_start(out_dram.ap()[:], acc[:])

            if passes == 1:
                body()
            else:
                with tc.For_i(0, passes):
                    body()
    nc.finalize()
    _NC_CACHE[passes] = nc
    return nc


def _host_prepare(beta, z0, v0, u, v, event_times, nu, nv, t0, tn):
    """Shard + gather inputs into per-core DMA-ready arrays.

    Host work is index gather, dtype cast, layout, and scalar-param
    replication only (plus the constant eps table shift)."""
    z0 = np.asarray(z0, dtype=np.float32)
    zue = (z0 + np.float32(EPS)).astype(BF16)   # u-side table with eps folded
    zb = z0.astype(BF16)
    vb = np.asarray(v0, dtype=np.float32).astype(BF16)
    zueu = zue.view(np.uint16)
    zbu = zb.view(np.uint16)
    vbu = vb.view(np.uint16)
    u = np.asarray(u).astype(np.int64, copy=False)
    v = np.asarray(v).astype(np.int64, copy=False)
    nu = np.asarray(nu).astype(np.int64, copy=False)
    nv = np.asarray(nv).astype(np.int64, copy=False)
    tarr = np.asarray(event_times, dtype=np.float32).astype(BF16).view(np.uint16)

    t0f = float(np.asarray(t0)); tnf = float(np.asarray(tn))
    dt = (tnf - t0f) / R
    taus = (t0f + (np.arange(R, dtype=np.float64) + 0.5) * dt).astype(np.float32)
    taus_arr = np.broadcast_to(taus[None, :], (128, R)).copy()
    betaf = float(np.asarray(beta).reshape(-1)[0])
    beta_arr = np.full((128, 1), betaf, dtype=np.float32)
    sent = np.array(PAD_SENTINEL, dtype=np.float32).astype(BF16).view(np.uint16).item()

    in_maps = []
    for c in range(N_CORES):
        es = slice(c * E_CORE, (c + 1) * E_CORE)
        ps = slice(c * P_CORE, (c + 1) * P_CORE)
        uc, vc = u[es], v[es]
        nuc, nvc = nu[ps], nv[ps]

        ev = np.zeros((NEV, E_PAD), dtype=np.uint16)
        ev[EV_S["zux"], :E_CORE] = zueu[uc, 0]
        ev[EV_S["zuy"], :E_CORE] = zueu[uc, 1]
        ev[EV_S["zvx"], :E_CORE] = zbu[vc, 0]
        ev[EV_S["zvy"], :E_CORE] = zbu[vc, 1]
        ev[EV_S["vux"], :E_CORE] = vbu[uc, 0]
        ev[EV_S["vuy"], :E_CORE] = vbu[uc, 1]
        ev[EV_S["vvx"], :E_CORE] = vbu[vc, 0]
        ev[EV_S["vvy"], :E_CORE] = vbu[vc, 1]
        ev[EV_S["t"], :E_CORE] = tarr[es]
        # [NEV, E_PAD] -> [EV_TILES, 128, NEV, EV_N]
        ev = ev.reshape(NEV, EV_TILES, 128, EV_N).transpose(1, 2, 0, 3).copy()

        pr = np.zeros((NPR, PR_N * 128), dtype=np.uint16)
        pr[PR_S["zvx"], P_CORE:] = sent
        pr[PR_S["zux"], :P_CORE] = zueu[nuc, 0]
        pr[PR_S["zuy"], :P_CORE] = zueu[nuc, 1]
        pr[PR_S["zvx"], :P_CORE] = zbu[nvc, 0]
        pr[PR_S["zvy"], :P_CORE] = zbu[nvc, 1]
        pr[PR_S["vux"], :P_CORE] = vbu[nuc, 0]
        pr[PR_S["vuy"], :P_CORE] = vbu[nuc, 1]
        pr[PR_S["vvx"], :P_CORE] = vbu[nvc, 0]
        pr[PR_S["vvy"], :P_CORE] = vbu[nvc, 1]
        pr = pr.reshape(NPR, 128, PR_N).transpose(1, 0, 2).copy()

        m = {"ev_all": ev.view(BF16), "pr_all": pr.view(BF16),
             "taus": taus_arr, "betab": beta_arr}
        in_maps.append(m)
    return in_maps, betaf, dt


def _combine(results, betaf, dt):
    d_sum = 0.0
    e_sum = 0.0
    for res in results:
        p = res["partials"].astype(np.float64)
        d_sum += p[:, 0:4].sum()
        e_sum += p[:, 4:4 + R].sum()
    val = N_EVENTS * float(betaf) - d_sum - e_sum * dt
    return np.array([[val]], dtype=np.float32)


def kernel(beta, z0, v0, u, v, event_times, nu, nv, t0, tn):
    from concourse import bass_utils
    in_maps, betaf, dt = _host_prepare(beta, z0, v0, u, v, event_times,
                                       nu, nv, t0, tn)
    nc = build_nc(passes=1)
    res = bass_utils.run_bass_kernel_spmd(nc, in_maps,
                                          core_ids=list(range(N_CORES)))
    return _combine(res.results, betaf, dt)
